# revision 18
# baseline (speedup 1.0000x reference)
"""Trainium2 Bass kernel for Performer-style (FAVOR+) causal linear attention.

Reference computation (per batch b=1, heads h=16, seq s=2048, d=64, r=64):
  qh = split_heads((q @ wq + bq) * d^-0.25)     kh likewise, vh = split_heads(v @ wv + bv)
  q' = (1/sqrt(d)) * exp(qh @ wg - 0.5*||qh||^2)   k' likewise
  attn[s] = (q'_s . sum_{j<=s} k'_j v_j^T) / (eps + q'_s . sum_{j<=s} k'_j)
  out = merge_heads(attn) @ wc + bc

Key simplifications:
  - wg is orthogonal (64x64 from QR), so ||qh||^2 == ||qh @ wg||^2. Folding
    wg into the projection weights (wqg = norm * wq @ blockdiag(wg)) means
    the kernel only computes qhg = q @ wqg.
  - The Q-side scalar prefactor exp(-0.5*||qhg||^2)/sqrt(d) cancels in the
    attn ratio (numerator and denominator share it; eps=1e-6 is negligible
    against the denominator), so q' := exp(qhg) comes from a single
    scalar-engine Exp. K keeps its prefactor (it sits inside the prefix
    sums); its 1/sqrt(d) also cancels and is dropped.
  - The causal scan is de-serialized: each chunk's state is an independent
    single matmul (both heads packed via the augmented-value layout);
    prefix states accumulate on the vector engine, with an fp16 copy on
    GpSimd feeding the inter-chunk matmuls.

Sharding: 2 heads per core (16 heads over 8 cores). Each core gets full
fp16 q/k/v (transposed) + its 128-column weight slices, computes its heads'
attention, projects through its 128-row slice of wc, and returns a
(2048, 1024) fp16 partial. The host sums the 8 partials and adds wc_b.
"""

import sys

if "/opt/trn_rl_repo" not in sys.path:
    sys.path.insert(0, "/opt/trn_rl_repo")

import math
from contextlib import ExitStack

import numpy as np

D_MODEL = 1024
N_HEADS = 16
D = 64  # head depth
R = 64  # kernel features (= D, wg orthogonal)
S = 2048
N_CORES = 8
HPC = N_HEADS // N_CORES  # heads per core = 2
CW = HPC * D  # per-core channel width = 128
P = 128
ST = 512  # projection s-tile width
NST = S // ST  # 4
C = 128  # scan chunk
NCH = S // C  # 16
KT = D_MODEL // P  # 8 contraction tiles
W = D + 1  # augmented value width (v | 1)
W2 = HPC * W  # 130
NORM_D = float(D ** (-0.25))

_CACHE = {}


def _build_bass():
    import os

    import concourse.bass as bass
    import concourse.mybir as mybir
    import concourse.tile as tile
    from concourse.bacc import Bacc

    # bisect flags: 1 = revert to baseline-style construct
    F_OUTDMA = int(os.environ.get("F_OUTDMA", "0"))  # per-chunk 2D out DMA
    F_ACT = int(os.environ.get("F_ACT", "0"))  # vector-engine feat/normalize

    f16 = mybir.dt.float16
    f32 = mybir.dt.float32
    AF = mybir.ActivationFunctionType
    Alu = mybir.AluOpType

    nc = Bacc(trn_type="TRN2")

    qT = nc.dram_tensor("qT", [D_MODEL, S], f16, kind="ExternalInput")
    kT = nc.dram_tensor("kT", [D_MODEL, S], f16, kind="ExternalInput")
    vT = nc.dram_tensor("vT", [D_MODEL, S], f16, kind="ExternalInput")
    # weights host-prearranged to [128, k*cw] so the DMA is flat
    wq = nc.dram_tensor("wq", [P, KT * CW], f16, kind="ExternalInput")
    wk = nc.dram_tensor("wk", [P, KT * CW], f16, kind="ExternalInput")
    wv = nc.dram_tensor("wv", [P, KT * CW], f16, kind="ExternalInput")
    # aux: [ident(128) | mask2(256) | ng(64)] packed along free dim
    aux = nc.dram_tensor("aux", [P, 3 * P + R], f16, kind="ExternalInput")
    bqkv = nc.dram_tensor("bqkv", [CW, 3], f32, kind="ExternalInput")
    wc = nc.dram_tensor("wc", [CW, D_MODEL], f16, kind="ExternalInput")
    out = nc.dram_tensor("out", [S, D_MODEL], f16, kind="ExternalOutput")

    with tile.TileContext(nc) as tc, ExitStack() as ctx:
        # ---- constant / weight / x tiles ----
        const = ctx.enter_context(tc.tile_pool(name="const", bufs=1))
        w_sb = {}
        for name, drt in (("wq", wq), ("wk", wk), ("wv", wv)):
            t = const.tile([P, KT * CW], f16, tag=name, name=f"wt_{name}")
            w_sb[name] = t
        b_all = const.tile([CW, 3], f32, tag="ball")
        b_sb = {"bq": b_all[:, 0:1], "bk": b_all[:, 1:2], "bv": b_all[:, 2:3]}
        aux_sb = const.tile([P, 3 * P + R], f16, tag="aux")
        id_sb = aux_sb[:, 0:P]
        mask2_sb = aux_sb[:, P : 3 * P]
        ng_sb = aux_sb[:, 3 * P : 3 * P + R]
        wc_sb = const.tile([CW, D_MODEL], f16, tag="wc")

        xin = ctx.enter_context(tc.tile_pool(name="xin", bufs=1))
        x_t = {}
        for name in ("q", "k", "v"):
            x_t[name] = xin.tile([P, KT * S], f16, tag=f"x_{name}", name=f"x_{name}")

        # persistent per-chunk V tiles ([v_h0|1|v_h1|1]) with ones at 64/129
        va_t = []
        for c in range(NCH):
            va = const.tile([P, W2], f16, tag=f"va{c}", name=f"va{c}")
            ones_ap = va[:].rearrange("p (b c) -> p b c", c=W)[:, :, D]
            nc.vector.memset(ones_ap, 1.0)
            va_t.append(va)
        # persistent per-chunk fp16 prefix tiles: the running prefix sum is
        # kept directly in fp16 (the vector-engine prefix add writes them),
        # feeding the inter-chunk matmuls with no extra copy
        p16_t = [None]
        for c in range(1, NCH):
            p16 = const.tile([P, W2], f16, tag=f"p16_{c}", name=f"p16_{c}")
            p16_t.append(p16)

        # ---- input DMA stream: weights interleaved with st0 halves, v,k,q ----
        def dma_x(name, srct, lo, hi):
            dst = x_t[name][:].rearrange("p (k s) -> p k s", k=KT)[:, :, lo:hi]
            sr = srct[:, lo:hi].rearrange("(k p) s -> p k s", p=P)
            nc.sync.dma_start(dst, sr)

        def dma_x_half(name, srct, st, half):
            h = KT // 2
            ks = slice(half * h, (half + 1) * h)
            sl = slice(st * ST, (st + 1) * ST)
            dst = x_t[name][:].rearrange("p (k s) -> p k s", k=KT)[:, ks, sl]
            sr = srct[:, sl].rearrange("(k p) s -> p k s", p=P)[:, ks, :]
            nc.sync.dma_start(dst, sr)

        nc.sync.dma_start(w_sb["wv"][:], wv[:, :])
        dma_x_half("v", vT, 0, 0)
        dma_x_half("v", vT, 0, 1)
        nc.sync.dma_start(w_sb["wk"][:], wk[:, :])
        dma_x_half("k", kT, 0, 0)
        dma_x_half("k", kT, 0, 1)
        nc.sync.dma_start(aux_sb[:], aux[:, :])
        nc.sync.dma_start(b_all[:], bqkv[:, :])
        nc.sync.dma_start(w_sb["wq"][:], wq[:, :])
        dma_x_half("q", qT, 0, 0)
        dma_x_half("q", qT, 0, 1)
        nc.sync.dma_start(wc_sb[:], wc[:, :])
        for st in range(1, NST):
            for name, srct in (("v", vT), ("k", kT), ("q", qT)):
                if st == NST - 1 and name == "q":
                    # split the tail-critical last q tile so its projection
                    # can start half a tile earlier
                    dma_x_half(name, srct, st, 0)
                    dma_x_half(name, srct, st, 1)
                else:
                    dma_x(name, srct, st * ST, (st + 1) * ST)

        def wslice(name, k):
            return w_sb["w" + name][:, k * CW : (k + 1) * CW]

        def xs(name, k, st):
            return x_t[name][:, k * S + st * ST : k * S + (st + 1) * ST]

        # ---- pools ----
        # PSUM banks (8): big(2: proj rotation + ng + fin) + tp(2) + so(2:
        # state+O ping-pong) + at(2: one per head — two accumulation groups
        # may NOT share a bank on HW, CoreSim accepts it but the device dies)
        big_psum = ctx.enter_context(tc.tile_pool(name="bigp", bufs=2, space="PSUM"))
        tp_psum = ctx.enter_context(tc.tile_pool(name="tpp", bufs=2, space="PSUM"))
        so_psum = ctx.enter_context(tc.tile_pool(name="sop", bufs=2, space="PSUM"))
        at_psum = ctx.enter_context(tc.tile_pool(name="atp", bufs=2, space="PSUM"))
        tmp_pool = ctx.enter_context(tc.tile_pool(name="tmp", bufs=2))
        qp_pool = ctx.enter_context(tc.tile_pool(name="qp", bufs=2))
        kp_pool = ctx.enter_context(tc.tile_pool(name="kp", bufs=2))
        vh_pool = ctx.enter_context(tc.tile_pool(name="vh", bufs=2))
        sc_pool = ctx.enter_context(tc.tile_pool(name="sc", bufs=3))
        ks_pool = ctx.enter_context(tc.tile_pool(name="ks", bufs=8))
        ot_pool = ctx.enter_context(tc.tile_pool(name="ot", bufs=2))
        ob_pool = ctx.enter_context(tc.tile_pool(name="obp", bufs=3))

        qp_t, kp_t, vh_t = [], [], []
        ks_t = [None] * NCH
        atm_t = [None] * NCH
        o_ps = [None] * NCH
        osb_t = [None] * NCH
        rc_t = [None] * NCH
        ob_t = [None] * (NCH // 2)

        def emit_proj(name, st):
            pp = big_psum.tile([P, ST], f32, tag="big", name=f"prj_{name}{st}")
            for k in range(KT):
                nc.tensor.matmul(
                    pp[:], wslice(name, k), xs(name, k, st),
                    start=(k == 0), stop=(k == KT - 1)
                )
            return pp

        def emit_post_v(pp, st):
            vh = vh_pool.tile([P, ST], f16, tag="vh")
            if F_ACT:
                nc.vector.tensor_scalar(vh[:], pp[:], b_sb["bv"][:], None, Alu.add)
            else:
                nc.scalar.activation(vh[:], pp[:], AF.Identity, bias=b_sb["bv"][:])
            vh_t.append(vh)

        def emit_feat_q(pp, st):
            # q' = exp(qhg + bq): prefactor cancels in the attn ratio
            pt = qp_pool.tile([P, ST], f16, tag="qkp")
            if F_ACT:
                tmp = tmp_pool.tile([P, ST], f16, tag="tmpq")
                nc.vector.tensor_scalar(tmp[:], pp[:], b_sb["bq"][:], None, Alu.add)
                nc.scalar.activation(pt[:], tmp[:], AF.Exp)
            else:
                nc.scalar.activation(pt[:], pp[:], AF.Exp, bias=b_sb["bq"][:])
            qp_t.append(pt)

        def emit_feat_k(pp, st):
            # k' = exp(khg + bk) * exp(-0.5*sum_d (khg+bk)^2); head-dim
            # reduction via quadrant-packed ng matmuls
            e1 = tmp_pool.tile([P, ST], f16, tag="e1k")
            tmp2 = tmp_pool.tile([P, ST], f16, tag="sqk")
            if F_ACT:
                tmp = tmp_pool.tile([P, ST], f16, tag="tmpk")
                nc.vector.tensor_scalar(tmp[:], pp[:], b_sb["bk"][:], None, Alu.add)
                nc.vector.tensor_tensor(tmp2[:], tmp[:], tmp[:], Alu.mult)
                nc.scalar.activation(e1[:], tmp[:], AF.Exp)
            else:
                nc.scalar.activation(e1[:], pp[:], AF.Exp, bias=b_sb["bk"][:])
                nc.scalar.activation(tmp2[:], pp[:], AF.Square, bias=b_sb["bk"][:])
            fp = big_psum.tile([P, ST], f32, tag="big", name=f"phi_k{st}")
            nc.tensor.matmul(fp[0:D, :], ng_sb[0:D, :], tmp2[0:D, :],
                             start=True, stop=True)
            nc.tensor.matmul(fp[D:P, :], ng_sb[D:P, :], tmp2[D:P, :],
                             start=True, stop=True, tile_position=(D, D))
            e2 = tmp_pool.tile([P, ST], f16, tag="e2k")
            nc.scalar.activation(e2[:], fp[:], AF.Exp)
            pt = kp_pool.tile([P, ST], f16, tag="qkp")
            nc.vector.tensor_tensor(pt[:], e1[:], e2[:], Alu.mult)
            kp_t.append(pt)

        def emit_tdma_v(st):
            for c in range(4 * st, 4 * st + 4):
                off = (c % 4) * C
                vtp = tp_psum.tile([P, P], f16, tag="tp", name=f"vtp{c}")
                nc.tensor.transpose(vtp[:], vh_t[st][:, off : off + C], id_sb[:])
                va_dst = va_t[c][:].rearrange("p (b c) -> p b c", c=W)[:, :, 0:D]
                nc.scalar.activation(
                    va_dst, vtp[:].rearrange("p (b c) -> p b c", c=D), AF.Copy
                )

        def emit_tdma_k(st):
            for c in range(4 * st, 4 * st + 4):
                off = (c % 4) * C
                ktp = tp_psum.tile([P, P], f16, tag="tp", name=f"ktp{c}")
                nc.tensor.transpose(ktp[:], kp_t[st][:, off : off + C], id_sb[:])
                ks = ks_pool.tile([P, P], f16, tag="ks", name=f"ks{c}")
                nc.vector.tensor_copy(ks[:], ktp[:])
                ks_t[c] = ks

        def emit_state(c):
            """Per-chunk state (one matmul, both heads) + fp16 prefix step."""
            if c + 1 >= NCH:
                return  # the last chunk's state is never read
            sp = so_psum.tile([P, W2], f32, tag="so", name=f"S{c}")
            nc.tensor.matmul(sp[:], ks_t[c][:], va_t[c][:], start=True, stop=True)
            if c == 0:
                nc.vector.tensor_copy(p16_t[1][:], sp[:])
            else:
                nc.vector.tensor_tensor(p16_t[c + 1][:], p16_t[c][:], sp[:], Alu.add)

        def emit_at(c):
            """Intra-chunk attention matrix, both heads in one bank + one mask."""
            st, off = c // 4, (c % 4) * C
            csl = slice(off, off + C)
            am = sc_pool.tile([P, 2 * P], f16, tag="atm", name=f"atm{c}")
            for h in range(HPC):
                atp = at_psum.tile([P, P], f32, tag="at", name=f"at{h}_{c}")
                nc.tensor.matmul(
                    atp[:], kp_t[st][h * D : (h + 1) * D, csl],
                    qp_t[st][h * D : (h + 1) * D, csl],
                    tile_position=(h * D, 0), start=True, stop=True,
                )
                nc.vector.tensor_tensor(
                    am[:, h * P : (h + 1) * P], atp[:], mask2_sb[:, 0:P],
                    Alu.mult,
                )
            atm_t[c] = am

        def emit_o_mm(c):
            """O = intra + inter matmuls, then batched recip + scalar scale."""
            st, off = c // 4, (c % 4) * C
            csl = slice(off, off + C)
            op_t = so_psum.tile([P, W2], f32, tag="so", name=f"o_{c}")
            for h in range(HPC):
                nc.tensor.matmul(
                    op_t[:, h * W : (h + 1) * W], atm_t[c][:, h * P : (h + 1) * P],
                    va_t[c][:, h * W : (h + 1) * W],
                    start=True, stop=(c == 0), skip_group_check=True,
                )
                if c > 0:
                    nc.tensor.matmul(
                        op_t[:, h * W : (h + 1) * W],
                        qp_t[st][h * D : (h + 1) * D, csl],
                        p16_t[c][h * D : (h + 1) * D, h * W : (h + 1) * W],
                        start=False, stop=True, skip_group_check=True,
                    )
            o_ps[c] = op_t
            rc = sc_pool.tile([P, HPC], f32, tag="rc")
            osb = sc_pool.tile([P, P], f16, tag="osb")
            den = op_t[:].rearrange("p (h w) -> p h w", w=W)[:, :, D]
            nc.vector.reciprocal(rc[:], den)
            # normalize: one head on scalar, one on vector
            nc.scalar.activation(
                osb[:, 0:D], op_t[:, 0:D], AF.Copy, scale=rc[:, 0:1]
            )
            nc.vector.tensor_scalar(
                osb[:, D:P], op_t[:, W : W + D], rc[:, 1:2], None, Alu.mult
            )
            rc_t[c] = rc
            osb_t[c] = osb

        ott_t = [None] * NCH

        def emit_fin_pre(c):
            """Transpose the normalized attn chunk back to hd-major."""
            otp = tp_psum.tile([P, P], f16, tag="tp", name=f"otp_{c}")
            nc.tensor.transpose(otp[:], osb_t[c][:], id_sb[:])
            ott = ot_pool.tile([P, P], f16, tag="ott")
            nc.vector.tensor_copy(ott[:], otp[:])
            ott_t[c] = ott

        def emit_fin_mm(c):
            """Final projection, stage fp16 output block, paired DMA."""
            ott = ott_t[c]
            if F_OUTDMA:
                ob = ob_pool.tile([P, D_MODEL], f16, tag="ob", name=f"ob{c}")
                col = 0
            else:
                pair, half = c // 2, c % 2
                if half == 0:
                    ob_t[pair] = ob_pool.tile(
                        [P, 2 * D_MODEL], f16, tag="ob", name=f"ob{pair}"
                    )
                ob = ob_t[pair]
                col = half * D_MODEL
            fps0 = big_psum.tile([P, ST], f32, tag="big", name=f"f0_{c}")
            nc.tensor.matmul(fps0[:], ott[:], wc_sb[:, 0:ST], start=True, stop=True)
            fps1 = big_psum.tile([P, ST], f32, tag="big", name=f"f1_{c}")
            nc.tensor.matmul(fps1[:], ott[:], wc_sb[:, ST:D_MODEL], start=True, stop=True)
            nc.scalar.activation(ob[:, col : col + ST], fps0[:], AF.Copy)
            nc.vector.tensor_copy(ob[:, col + ST : col + D_MODEL], fps1[:])
            if F_OUTDMA:
                nc.scalar.dma_start(out[c * C : (c + 1) * C, :], ob[:])
            elif half == 1:
                dst = out[2 * pair * C : (2 * pair + 2) * C, :].rearrange(
                    "(b p) m -> p b m", p=P
                )
                src = ob[:].rearrange("p (b m) -> p b m", m=D_MODEL)
                nc.scalar.dma_start(dst, src)

        fin_done = [0]  # next chunk whose fin is pending

        def emit_chunks(st):
            # per-chunk tensor order hides every cross-engine wait: the
            # previous chunk's transpose + finals stream between this chunk's
            # AT/state matmuls and its O matmuls, so the vector-engine mask /
            # prefix / normalize results are ready when the tensor needs them.
            for c in range(4 * st, 4 * st + 4):
                f = fin_done[0] if fin_done[0] < c else -1
                if f >= 0:
                    emit_fin_pre(f)
                emit_at(c)
                emit_state(c)
                if f >= 0:
                    emit_fin_mm(f)
                    fin_done[0] += 1
                emit_o_mm(c)

        # ---- interleaved emission: chunks(st-1) | proj/feat(st) ----
        for st in range(NST):
            if st > 0:
                emit_chunks(st - 1)
            pv = emit_proj("v", st)
            emit_post_v(pv, st)
            pk = emit_proj("k", st)
            emit_feat_k(pk, st)
            emit_tdma_v(st)
            pq = emit_proj("q", st)
            emit_feat_q(pq, st)
            emit_tdma_k(st)
        emit_chunks(NST - 1)
        while fin_done[0] < NCH:
            emit_fin_pre(fin_done[0])
            emit_fin_mm(fin_done[0])
            fin_done[0] += 1

    nc.finalize()
    return nc


def _prep_inputs(v, k, q, wq_w, wq_b, wk_w, wk_b, wv_w, wv_b, wc_w, wc_b, wg):
    f16 = np.float16
    qT = np.ascontiguousarray(q[0].T).astype(f16)
    kT = np.ascontiguousarray(k[0].T).astype(f16)
    vT = np.ascontiguousarray(v[0].T).astype(f16)
    ident = np.eye(P, dtype=f16)
    mask = np.triu(np.ones((P, P), np.float32)).astype(f16)  # mask[j,i]=1 iff j<=i
    ng = np.full((P, R), -0.5, f16)
    aux = np.concatenate([ident, mask, mask, ng], axis=1)  # (128, 448)

    # fold wg into q/k projections (wg orthogonal: ||x@wg|| == ||x||)
    wg32 = wg.astype(np.float32)
    def fold(w, b):
        wf = np.zeros((D_MODEL, D_MODEL), np.float32)
        bf = np.zeros(D_MODEL, np.float32)
        for h in range(N_HEADS):
            sl = slice(h * D, (h + 1) * D)
            wf[:, sl] = (w[:, sl].astype(np.float32) * NORM_D) @ wg32
            bf[sl] = (b[sl].astype(np.float32) * NORM_D) @ wg32
        return wf, bf
    wqg, bqg = fold(wq_w, wq_b)
    wkg, bkg = fold(wk_w, wk_b)

    def warr(w):  # [1024, 128] -> [128, 8*128] with [p, k*128+c] = w[k*128+p, c]
        return np.ascontiguousarray(
            w.reshape(KT, P, CW).transpose(1, 0, 2).reshape(P, KT * CW)
        ).astype(f16)

    in_maps = []
    for c in range(N_CORES):
        cs = slice(c * CW, (c + 1) * CW)
        bqkv = np.stack([
            bqg[cs].astype(np.float32),
            bkg[cs].astype(np.float32),
            wv_b[cs].astype(np.float32),
        ], axis=1)
        in_maps.append({
            "qT": qT, "kT": kT, "vT": vT,
            "wq": warr(wqg[:, cs]),
            "wk": warr(wkg[:, cs]),
            "wv": warr(wv_w[:, cs].astype(np.float32)),
            "bqkv": bqkv,
            "aux": aux,
            "wc": wc_w[cs, :].astype(f16),
        })
    return in_maps


def kernel(**inputs):
    from concourse.bass_utils import run_bass_kernel_spmd

    if "nc" not in _CACHE:
        _CACHE["nc"] = _build_bass()
    nc = _CACHE["nc"]
    in_maps = _prep_inputs(**inputs)
    res = run_bass_kernel_spmd(nc, in_maps, core_ids=list(range(N_CORES)))
    _CACHE["last_results"] = res
    acc = np.zeros((S, D_MODEL), np.float32)
    for c in range(N_CORES):
        acc += res.results[c]["out"].astype(np.float32)
    acc += inputs["wc_b"].astype(np.float32)[None, :]
    return acc[None, :, :]


if __name__ == "__main__":
    import reference

    inp = {k: np.asarray(v) for k, v in reference.setup_inputs().items()}
    got = kernel(**inp)
    print("kernel out", got.shape, got.dtype)


# revision 20
# speedup vs baseline: 1.0800x; 1.0800x over previous
"""Trainium2 Bass kernel for Performer-style (FAVOR+) causal linear attention.

Reference computation (per batch b=1, heads h=16, seq s=2048, d=64, r=64):
  qh = split_heads((q @ wq + bq) * d^-0.25)     kh likewise, vh = split_heads(v @ wv + bv)
  q' = (1/sqrt(d)) * exp(qh @ wg - 0.5*||qh||^2)   k' likewise
  attn[s] = (q'_s . sum_{j<=s} k'_j v_j^T) / (eps + q'_s . sum_{j<=s} k'_j)
  out = merge_heads(attn) @ wc + bc

Key simplifications:
  - wg is orthogonal (64x64 from QR), so ||qh||^2 == ||qh @ wg||^2. Folding
    wg into the projection weights (wqg = norm * wq @ blockdiag(wg)) means
    the kernel only computes qhg = q @ wqg.
  - The Q-side scalar prefactor exp(-0.5*||qhg||^2)/sqrt(d) cancels in the
    attn ratio (numerator and denominator share it; eps=1e-6 is negligible
    against the denominator), so q' := exp(qhg) needs no sum-of-squares
    reduction at all. K keeps its prefactor (it sits inside the prefix sums).
  - The causal scan is de-serialized: each chunk's state is an independent
    single matmul (both heads packed via the augmented-value layout). The
    running prefix is accumulated directly in fp16 on the vector engine,
    writing only the block-diagonal head blocks; the off-diagonal blocks
    stay zero so the inter-chunk contribution is a single 128-contraction
    matmul per chunk.

Sharding: 2 heads per core (16 heads over 8 cores). Each core gets full
fp16 q/k/v (transposed) + its 128-column weight slices, computes its heads'
attention, projects through its 128-row slice of wc, and returns a
(2048, 1024) fp16 partial. The host sums the 8 partials and adds wc_b.
"""

import sys

if "/opt/trn_rl_repo" not in sys.path:
    sys.path.insert(0, "/opt/trn_rl_repo")

import math
from contextlib import ExitStack

import numpy as np

D_MODEL = 1024
N_HEADS = 16
D = 64  # head depth
R = 64  # kernel features (= D, wg orthogonal)
S = 2048
N_CORES = 8
HPC = N_HEADS // N_CORES  # heads per core = 2
CW = HPC * D  # per-core channel width = 128
P = 128
ST = 512  # projection s-tile width
NST = S // ST  # 4
C = 128  # scan chunk
NCH = S // C  # 16
KT = D_MODEL // P  # 8 contraction tiles
W = D + 1  # augmented value width (v | 1)
W2 = HPC * W  # 130
NORM_D = float(D ** (-0.25))
LN_RSQRT_D = float(-0.5 * math.log(D))  # exp(x + this) = exp(x)/sqrt(d)

_CACHE = {}


def _build_bass():
    import concourse.bass as bass
    import concourse.mybir as mybir
    import concourse.tile as tile
    from concourse.bacc import Bacc

    f16 = mybir.dt.float16
    f32 = mybir.dt.float32
    AF = mybir.ActivationFunctionType
    Alu = mybir.AluOpType

    nc = Bacc(trn_type="TRN2")

    qT = nc.dram_tensor("qT", [D_MODEL, S], f16, kind="ExternalInput")
    kT = nc.dram_tensor("kT", [D_MODEL, S], f16, kind="ExternalInput")
    vT = nc.dram_tensor("vT", [D_MODEL, S], f16, kind="ExternalInput")
    # weights host-prearranged to [128, k*cw] so the DMA is flat
    wq = nc.dram_tensor("wq", [P, KT * CW], f16, kind="ExternalInput")
    wk = nc.dram_tensor("wk", [P, KT * CW], f16, kind="ExternalInput")
    wv = nc.dram_tensor("wv", [P, KT * CW], f16, kind="ExternalInput")
    # aux: [ident(128) | mask(128) | ng(64)] packed along free dim
    aux = nc.dram_tensor("aux", [P, 2 * P + R], f16, kind="ExternalInput")
    bqkv = nc.dram_tensor("bqkv", [CW, 3], f32, kind="ExternalInput")
    wc = nc.dram_tensor("wc", [CW, D_MODEL], f16, kind="ExternalInput")
    out = nc.dram_tensor("out", [S, D_MODEL], f16, kind="ExternalOutput")

    with tile.TileContext(nc) as tc, ExitStack() as ctx:
        # ---- constant / weight tiles (sync queue, ahead of the x stream) ----
        const = ctx.enter_context(tc.tile_pool(name="const", bufs=1))
        w_sb = {}
        for name, drt in (("wq", wq), ("wk", wk), ("wv", wv)):
            t = const.tile([P, KT * CW], f16, tag=name, name=f"wt_{name}")
            nc.sync.dma_start(t[:], drt[:, :])
            for k in range(KT):
                w_sb[(name, k)] = t[:, k * CW : (k + 1) * CW]
        b_all = const.tile([CW, 3], f32, tag="ball")
        nc.sync.dma_start(b_all[:], bqkv[:, :])
        b_sb = {"bq": b_all[:, 0:1], "bk": b_all[:, 1:2], "bv": b_all[:, 2:3]}
        aux_sb = const.tile([P, 2 * P + R], f16, tag="aux")
        nc.sync.dma_start(aux_sb[:], aux[:, :])
        id_sb = aux_sb[:, 0:P]
        mask_sb = aux_sb[:, P : 2 * P]
        ng_sb = aux_sb[:, 2 * P : 2 * P + R]
        wc_sb = const.tile([CW, D_MODEL], f16, tag="wc")
        ebias = const.tile([P, 1], f32, tag="ebias")
        nc.vector.memset(ebias[:], LN_RSQRT_D)

        # persistent per-chunk V tiles ([v_h0|1|v_h1|1]) with ones at 64/129
        va_t = []
        for c in range(NCH):
            va = const.tile([P, W2], f16, tag=f"va{c}", name=f"va{c}")
            ones_ap = va[:].rearrange("p (b c) -> p b c", c=W)[:, :, D]
            nc.vector.memset(ones_ap, 1.0)
            va_t.append(va)
        # persistent per-chunk fp16 prefix tiles (zeroed once): the prefix
        # add writes only the diagonal head blocks, so the off-diagonal
        # blocks stay zero and the O-inter matmul can contract all 128
        # partitions in one shot.
        p16_t = [None]
        for c in range(1, NCH):
            p16 = const.tile([P, W2], f16, tag=f"p16_{c}", name=f"p16_{c}")
            nc.vector.memset(p16[:], 0.0)
            p16_t.append(p16)

        # ---- x input tiles, DMA'd st-major: (q,k,v) x st, 1MB per DMA ----
        xin = ctx.enter_context(tc.tile_pool(name="xin", bufs=1))
        x_t = {}
        for name in ("q", "k", "v"):
            x_t[name] = xin.tile([P, KT * S], f16, tag=f"x_{name}", name=f"x_{name}")
        for st in range(NST):
            sl = slice(st * ST, (st + 1) * ST)
            for name, srct in (("q", qT), ("k", kT), ("v", vT)):
                dst = x_t[name][:].rearrange("p (k s) -> p k s", k=KT)[:, :, sl]
                sr = srct[:, sl].rearrange("(k p) s -> p k s", p=P)
                if st == 0 and name in ("q", "k"):
                    # split into k-tile halves so the first projections can
                    # start as soon as half the s-tile has landed
                    h = KT // 2
                    nc.sync.dma_start(dst[:, 0:h, :], sr[:, 0:h, :])
                    nc.sync.dma_start(dst[:, h:KT, :], sr[:, h:KT, :])
                else:
                    nc.sync.dma_start(dst, sr)
            if st == 0:  # wc is first needed ~25us in; don't delay the x stream
                nc.sync.dma_start(wc_sb[:], wc[:, :])

        def xs(name, k, st):
            return x_t[name][:, k * S + st * ST : k * S + (st + 1) * ST]

        # ---- pools ----
        tmp_pool = ctx.enter_context(tc.tile_pool(name="tmp", bufs=3))
        # PSUM: 8 banks x 2KB/partition: bigp(3) + tpp(2) + sp(1) + atp(1) + op(1)
        big_psum = ctx.enter_context(tc.tile_pool(name="bigp", bufs=2, space="PSUM"))
        tp_psum = ctx.enter_context(tc.tile_pool(name="tpp", bufs=2, space="PSUM"))
        s_psum = ctx.enter_context(tc.tile_pool(name="sp", bufs=2, space="PSUM"))
        at_psum = ctx.enter_context(tc.tile_pool(name="atp", bufs=1, space="PSUM"))
        o_psum = ctx.enter_context(tc.tile_pool(name="op", bufs=1, space="PSUM"))
        qp_pool = ctx.enter_context(tc.tile_pool(name="qp", bufs=NST))
        kp_pool = ctx.enter_context(tc.tile_pool(name="kp", bufs=NST))
        vh_pool = ctx.enter_context(tc.tile_pool(name="vh", bufs=NST))
        sc_pool = ctx.enter_context(tc.tile_pool(name="sc", bufs=4))
        ks_pool = ctx.enter_context(tc.tile_pool(name="ks", bufs=8))
        ot_pool = ctx.enter_context(tc.tile_pool(name="ot", bufs=3))
        ob_pool = ctx.enter_context(tc.tile_pool(name="obp", bufs=3))

        qp_t, kp_t, vh_t = [], [], []
        ks_t = [None] * NCH
        s_ps = [None] * NCH
        atm_t = [None] * NCH
        ob_t = [None] * (NCH // 2)

        def emit_proj(name, st):
            pp = big_psum.tile([P, ST], f32, tag="big", name=f"prj_{name}{st}")
            for k in range(KT):
                nc.tensor.matmul(
                    pp[:], w_sb[("w" + name, k)][:], xs(name, k, st),
                    start=(k == 0), stop=(k == KT - 1)
                )
            return pp

        def emit_post_v(pp, st):
            vh = vh_pool.tile([P, ST], f16, tag="vh")
            nc.vector.tensor_scalar(vh[:], pp[:], b_sb["bv"][:], None, Alu.add)
            vh_t.append(vh)

        def emit_feat_q(pp, st):
            """q' = exp(qhg + bq): the scalar prefactor cancels in the
            attn ratio, so no sum-of-squares reduction is needed."""
            tmp = tmp_pool.tile([P, ST], f16, tag="tmpl_q")
            nc.vector.tensor_scalar(tmp[:], pp[:], b_sb["bq"][:], None, Alu.add)
            pt = qp_pool.tile([P, ST], f16, tag="qkp")
            nc.scalar.activation(pt[:], tmp[:], AF.Exp)
            qp_t.append(pt)

        def emit_feat_k(pp, st):
            """k' = exp(khg) * exp(-0.5*sum_d khg^2 + ln(1/sqrt d)).
            Head-dim reduction via quadrant-packed ng matmuls."""
            tmp = tmp_pool.tile([P, ST], f16, tag="tmpl_k")
            nc.vector.tensor_scalar(tmp[:], pp[:], b_sb["bk"][:], None, Alu.add)
            tmp2 = tmp_pool.tile([P, ST], f16, tag="tmps_k")
            nc.vector.tensor_tensor(tmp2[:], tmp[:], tmp[:], Alu.mult)
            fp = big_psum.tile([P, ST], f32, tag="big", name=f"phi_k{st}")
            nc.tensor.matmul(fp[0:D, :], ng_sb[0:D, :], tmp2[0:D, :],
                             start=True, stop=True)
            nc.tensor.matmul(fp[D:P, :], ng_sb[D:P, :], tmp2[D:P, :],
                             start=True, stop=True, tile_position=(D, D))
            e1 = tmp_pool.tile([P, ST], f16, tag="e1_k")
            nc.scalar.activation(e1[:], tmp[:], AF.Exp)
            e2 = tmp_pool.tile([P, ST], f16, tag="e2_k")
            nc.scalar.activation(e2[:], fp[:], AF.Exp, bias=ebias[:])
            pt = kp_pool.tile([P, ST], f16, tag="qkp")
            nc.vector.tensor_tensor(pt[:], e1[:], e2[:], Alu.mult)
            kp_t.append(pt)

        def emit_tdma(st):
            """PE transposes of k' and v chunks to s-major."""
            for c in range(4 * st, 4 * st + 4):
                off = (c % 4) * C
                csl = slice(off, off + C)
                ktp = tp_psum.tile([P, P], f16, tag="tp", name=f"ktp{c}")
                nc.tensor.transpose(ktp[:], kp_t[st][:, csl], id_sb[:])
                ks = ks_pool.tile([P, P], f16, tag="ks", name=f"ks{c}")
                nc.vector.tensor_copy(ks[:], ktp[:])
                ks_t[c] = ks
                vtp = tp_psum.tile([P, P], f16, tag="tp", name=f"vtp{c}")
                nc.tensor.transpose(vtp[:], vh_t[st][:, csl], id_sb[:])
                va_dst = va_t[c][:].rearrange("p (b c) -> p b c", c=W)[:, :, 0:D]
                nc.scalar.activation(
                    va_dst, vtp[:].rearrange("p (b c) -> p b c", c=D), AF.Copy
                )

        def emit_state(c):
            """Per-chunk state (one matmul, both heads) + fp16 prefix step
            writing only the diagonal head blocks."""
            if c + 1 >= NCH:
                return  # the last chunk's state is never read
            sp = s_psum.tile([P, W2], f32, tag="S", name=f"S{c}")
            nc.tensor.matmul(sp[:], ks_t[c][:], va_t[c][:], start=True, stop=True)
            s_ps[c] = sp
            for h in range(HPC):
                rsl = slice(h * D, (h + 1) * D)
                csl = slice(h * W, (h + 1) * W)
                if c == 0:
                    nc.vector.tensor_copy(p16_t[1][rsl, csl], sp[rsl, csl])
                else:
                    nc.vector.tensor_tensor(
                        p16_t[c + 1][rsl, csl], p16_t[c][rsl, csl], sp[rsl, csl],
                        Alu.add,
                    )

        def emit_at(c):
            """Intra-chunk attention matrix + mask (two tiles, one bank)."""
            st, off = c // 4, (c % 4) * C
            csl = slice(off, off + C)
            atm = []
            for h in range(HPC):
                atp = at_psum.tile([P, P], f32, tag="at", name=f"at{h}_{c}")
                nc.tensor.matmul(
                    atp[:], kp_t[st][h * D : (h + 1) * D, csl],
                    qp_t[st][h * D : (h + 1) * D, csl],
                    tile_position=(h * D, 0), start=True, stop=True,
                )
                am = sc_pool.tile([P, P], f16, tag=f"atm{h}", name=f"atm{h}_{c}")
                nc.vector.tensor_tensor(am[:], atp[:], mask_sb[:], Alu.mult)
                atm.append(am)
            atm_t[c] = atm

        o_ps = [None] * NCH
        osb_t = [None] * NCH

        def emit_o_mm(c):
            """O = intra matmuls + single block-diagonal inter matmul, then
            batched reciprocal + normalize."""
            st, off = c // 4, (c % 4) * C
            csl = slice(off, off + C)
            va = va_t[c]
            op_t = o_psum.tile([P, W2], f32, tag="o", name=f"o_{c}")
            if c > 0:
                # p16 off-diagonal blocks are zero, so both heads' inter
                # contributions come from one 128-contraction matmul; it
                # starts the accumulation group (zeroing the whole region)
                # and doesn't need the masked AT, so it runs early
                nc.tensor.matmul(
                    op_t[:], qp_t[st][:, csl], p16_t[c][:],
                    start=True, stop=False, skip_group_check=True,
                )
            for h in range(HPC):
                nc.tensor.matmul(
                    op_t[:, h * W : (h + 1) * W], atm_t[c][h][:],
                    va[:, h * W : (h + 1) * W],
                    start=(c == 0), stop=True, skip_group_check=True,
                )
            o_ps[c] = op_t
            rc = sc_pool.tile([P, HPC], f32, tag="rc")
            den = op_t[:].rearrange("p (h w) -> p h w", w=W)[:, :, D]
            nc.vector.reciprocal(rc[:], den)
            osb = sc_pool.tile([P, P], f16, tag="osb")
            for h in range(HPC):
                nc.vector.tensor_scalar(
                    osb[:, h * D : (h + 1) * D], op_t[:, h * W : h * W + D],
                    rc[:, h : h + 1], None, Alu.mult,
                )
            osb_t[c] = osb

        def emit_fin(c):
            """Transpose back, final projection, paired store."""
            otp = tp_psum.tile([P, P], f16, tag="tp", name=f"otp_{c}")
            nc.tensor.transpose(otp[:], osb_t[c][:], id_sb[:])
            ott = ot_pool.tile([P, P], f16, tag="ott")
            nc.vector.tensor_copy(ott[:], otp[:])
            pair, half = c // 2, c % 2
            if half == 0:
                ob_t[pair] = ob_pool.tile(
                    [P, 2 * D_MODEL], f16, tag="ob", name=f"ob{pair}"
                )
            ob = ob_t[pair]
            col = half * D_MODEL
            fps0 = big_psum.tile([P, ST], f32, tag="big", name=f"f0_{c}")
            nc.tensor.matmul(fps0[:], ott[:], wc_sb[:, 0:ST], start=True, stop=True)
            fps1 = big_psum.tile([P, ST], f32, tag="big", name=f"f1_{c}")
            nc.tensor.matmul(fps1[:], ott[:], wc_sb[:, ST:D_MODEL], start=True, stop=True)
            nc.scalar.activation(ob[:, col : col + ST], fps0[:], AF.Copy)
            nc.vector.tensor_copy(ob[:, col + ST : col + D_MODEL], fps1[:])
            if half == 1:
                dst = out[2 * pair * C : (2 * pair + 2) * C, :].rearrange(
                    "(b p) m -> p b m", p=P
                )
                src = ob[:].rearrange("p (b m) -> p b m", m=D_MODEL)
                nc.scalar.dma_start(dst, src)

        fin_done = [0]  # next chunk whose fin is pending

        def emit_chunks(st):
            # fin(c-1) is emitted between chunk c's AT and O matmuls: the
            # tensor engine streams otp+finals of the previous chunk while
            # the vector engine masks this chunk's AT.
            for c in range(4 * st, 4 * st + 4):
                emit_state(c)
                emit_at(c)
                while fin_done[0] < c:
                    emit_fin(fin_done[0])
                    fin_done[0] += 1
                emit_o_mm(c)

        # ---- interleaved emission: proj(st) | feat(st) | chunks(st-1) ----
        for st in range(NST):
            pq = emit_proj("q", st)
            pk = emit_proj("k", st)
            emit_feat_q(pq, st)
            emit_feat_k(pk, st)
            pv = emit_proj("v", st)
            emit_post_v(pv, st)
            emit_tdma(st)
            if st > 0:
                emit_chunks(st - 1)
        emit_chunks(NST - 1)
        while fin_done[0] < NCH:
            emit_fin(fin_done[0])
            fin_done[0] += 1

    nc.finalize()
    return nc


def _prep_inputs(v, k, q, wq_w, wq_b, wk_w, wk_b, wv_w, wv_b, wc_w, wc_b, wg):
    f16 = np.float16
    qT = np.ascontiguousarray(q[0].T).astype(f16)
    kT = np.ascontiguousarray(k[0].T).astype(f16)
    vT = np.ascontiguousarray(v[0].T).astype(f16)
    ident = np.eye(P, dtype=f16)
    mask = np.triu(np.ones((P, P), np.float32)).astype(f16)  # mask[j,i]=1 iff j<=i
    ng = np.full((P, R), -0.5, f16)
    aux = np.concatenate([ident, mask, ng], axis=1)  # (128, 320)

    # fold wg into q/k projections (wg orthogonal: ||x@wg|| == ||x||)
    wg32 = wg.astype(np.float32)
    def fold(w, b):
        wf = np.zeros((D_MODEL, D_MODEL), np.float32)
        bf = np.zeros(D_MODEL, np.float32)
        for h in range(N_HEADS):
            sl = slice(h * D, (h + 1) * D)
            wf[:, sl] = (w[:, sl].astype(np.float32) * NORM_D) @ wg32
            bf[sl] = (b[sl].astype(np.float32) * NORM_D) @ wg32
        return wf, bf
    wqg, bqg = fold(wq_w, wq_b)
    wkg, bkg = fold(wk_w, wk_b)

    def warr(w):  # [1024, 128] -> [128, 8*128] with [p, k*128+c] = w[k*128+p, c]
        return np.ascontiguousarray(
            w.reshape(KT, P, CW).transpose(1, 0, 2).reshape(P, KT * CW)
        ).astype(f16)

    in_maps = []
    for c in range(N_CORES):
        cs = slice(c * CW, (c + 1) * CW)
        bqkv = np.stack([
            bqg[cs].astype(np.float32),
            bkg[cs].astype(np.float32),
            wv_b[cs].astype(np.float32),
        ], axis=1)
        in_maps.append({
            "qT": qT, "kT": kT, "vT": vT,
            "wq": warr(wqg[:, cs]),
            "wk": warr(wkg[:, cs]),
            "wv": warr(wv_w[:, cs].astype(np.float32)),
            "bqkv": bqkv,
            "aux": aux,
            "wc": wc_w[cs, :].astype(f16),
        })
    return in_maps


def kernel(**inputs):
    from concourse.bass_utils import run_bass_kernel_spmd

    if "nc" not in _CACHE:
        _CACHE["nc"] = _build_bass()
    nc = _CACHE["nc"]
    in_maps = _prep_inputs(**inputs)
    res = run_bass_kernel_spmd(nc, in_maps, core_ids=list(range(N_CORES)))
    _CACHE["last_results"] = res
    acc = np.zeros((S, D_MODEL), np.float32)
    for c in range(N_CORES):
        acc += res.results[c]["out"].astype(np.float32)
    acc += inputs["wc_b"].astype(np.float32)[None, :]
    return acc[None, :, :]


if __name__ == "__main__":
    import reference

    inp = {k: np.asarray(v) for k, v in reference.setup_inputs().items()}
    got = kernel(**inp)
    print("kernel out", got.shape, got.dtype)
